# revision 29
# baseline (speedup 1.0000x reference)
"""Trainium2 Bass kernel for the LocalizeModule retrieval problem.

Computation (reference):
    f  = relu(feat @ W1.T + b1) @ W2.T + b2        # [F, H]
    k  = keyword @ Wk.T + bk                       # [K, H]
    out = (cos_sim(k, f) + 1) * 0.49               # [K, F]

Sharding across 8 cores (v2):
  * frames (F) sharded for the MLP: each core projects its F/8 frames;
  * keywords (K) sharded for the keyword projection: each core projects
    K/8 keywords, normalizes them (k-hat * 64), quantizes to fp8-e4m3,
    and AllGathers the full normalized keyword matrix (0.5 MB/rank on
    the TOPSP/SDMA collective path, fully overlapped with the MLP);
  * score GEMM per core: [FS, K] = fT.T @ k8_all in fp8 DoubleRow with
    the frame tile stationary (best LDWEIGHTS amortization).  Output is
    the TRANSPOSED score shard [FS, K]; the host concatenates out_c.T
    along F.

On-chip layout keeps H on partitions throughout so all matmuls contract
over the partition dim.  Frame norms land partition-major via a
ones-vector matmul, so the whole score epilogue is one ScalarE op per
PSUM bank: out = rfn * psum + 0.49 with rfn = (0.49/64)/max(||f||,eps).
"""

import numpy as np
import ml_dtypes

import concourse.bass as bass  # noqa: F401  (bass types used via tile/bacc)
import concourse.mybir as mybir
import concourse.tile as tile
from concourse import bacc
from concourse.bass_utils import run_bass_kernel_spmd

P = 128
H = 1024
F = 8192
K = 4096
NCORES = 8
FS = F // NCORES          # 1024 frames per core
KS = K // NCORES          # 512 keywords per core
HO = H // P               # 8 partition chunks of the hidden dim
NCH = 512                 # matmul moving/free chunk (one PSUM bank of fp32)
F_CHUNKS = FS // NCH      # 2
F_TILES = FS // P         # 8
K_CHUNKS = K // NCH       # 8
EPS = 1e-8
OUT_SCALE = 0.49
KSCALE = 64.0             # pre-scale of normalized keywords into fp8 range

BF16 = mybir.dt.bfloat16
FP8 = mybir.dt.float8e4
F32 = mybir.dt.float32
AF = mybir.ActivationFunctionType
ALU = mybir.AluOpType

_CACHE = {}

LAST_EXEC_NS = None
LAST_RESULTS = None


def _emit(tc, io):
    nc = tc.nc
    featT_d, kwT_d, w1t_d, w2t_d, wkt_d, b1_d, b2_d, bk_d, out_d = io

    import contextlib

    with contextlib.ExitStack() as ctx:
        const = ctx.enter_context(tc.tile_pool(name="const", bufs=1))
        psum = ctx.enter_context(tc.tile_pool(name="psum", bufs=1, space="PSUM"))
        dram = ctx.enter_context(tc.tile_pool(name="dram", bufs=1, space="DRAM"))

        # ---- persistent SBUF tensors -------------------------------------
        wkt_s = const.tile([P, HO, H], BF16)
        w2t_s = const.tile([P, HO, H], BF16)
        b1_s = const.tile([P, HO], F32)
        b2_s = const.tile([P, HO], F32)
        bk_s = const.tile([P, HO], F32)
        ones_s = const.tile([P, 1], BF16)
        bias049_s = const.tile([P, 1], F32)
        kbf_s = const.tile([P, HO, KS], BF16)     # projected keyword shard
        k8_all = const.tile([P, HO, K], FP8)      # gathered normalized keywords
        hT_s = const.tile([P, HO, FS], BF16)      # relu(W1 @ featT + b1)
        f8_s = const.tile([P, HO, FS], FP8)       # projected frames, fp8
        rkn_b = const.tile([P, KS], F32)          # KSCALE/||k||, bcast on partitions
        nf_raw = const.tile([P, F_TILES], F32)    # ||f||^2, partition-major
        rfn_p = const.tile([P, F_TILES], F32)     # (0.49/KSCALE)/||f||, partition-major

        nc.vector.memset(bias049_s[:], OUT_SCALE)
        nc.vector.memset(ones_s[:], 1.0)

        # DRAM bounce buffers for the keyword AllGather
        cc_in = dram.tile([P, HO, KS], FP8)
        cc_out = dram.tile([NCORES, P, HO, KS], FP8, addr_space="Shared")
        # load-failure probes: 1-D DRAM tile + bf16 partition_broadcast
        probe_dram = dram.tile([KS], BF16)
        probe_2d = const.tile([P, KS // P], BF16)
        nc.vector.memset(probe_2d[:], 1.0)
        nc.sync.dma_start(probe_dram[:], probe_2d[:])

        def mm_accum(ps, lhs_t, lhs_sl, rhs_t, rhs_sl):
            for ho in range(HO):
                nc.tensor.matmul(
                    ps, lhs_t[:, ho, lhs_sl], rhs_t[:, ho, rhs_sl],
                    start=(ho == 0), stop=(ho == HO - 1),
                )

        with tc.tile_pool(name="work", bufs=1) as work:

            def emit_tree_sum(sq, width):
                """Pairwise-tree DVE sum of sq[:, ho, :] over the HO axis."""
                tmps = []
                for i in range(HO // 2):
                    tmp = work.tile([P, width], BF16, tag="tsum", bufs=4, name="tsum")
                    nc.vector.tensor_tensor(
                        tmp[:], sq[:, 2 * i, :width], sq[:, 2 * i + 1, :width], ALU.add
                    )
                    tmps.append(tmp)
                nc.vector.tensor_tensor(tmps[0][:], tmps[0][:], tmps[1][:], ALU.add)
                nc.vector.tensor_tensor(tmps[2][:], tmps[2][:], tmps[3][:], ALU.add)
                ssum = work.tile([P, width], BF16, tag="sqs", bufs=3, name="ssum")
                nc.vector.tensor_tensor(ssum[:], tmps[0][:], tmps[2][:], ALU.add)
                return ssum

            # ---- phase K: keyword projection on this core's K/8 shard ----
            kw_s = work.tile([P, HO, KS], BF16, tag="kw", bufs=1, name="kw_s")
            sqk_s = work.tile([P, HO, KS], BF16, tag="sqk", bufs=1, name="sqk_s")
            for ho in range(HO):
                nc.sync.dma_start(kw_s[:, ho], kwT_d[:, ho])
                nc.sync.dma_start(wkt_s[:, ho], wkt_d[:, ho])
            nc.sync.dma_start(bk_s[:], bk_d[:])
            nc.sync.dma_start(b1_s[:], b1_d[:])
            nc.sync.dma_start(b2_s[:], b2_d[:])

            with tc.tile_pool(name="mlp_in", bufs=1) as mlp_in:
                featT_s = mlp_in.tile([P, HO, FS], BF16)
                w1t_s = mlp_in.tile([P, HO, H], BF16)
                # MLP1 inputs queue behind the keyword-projection inputs
                for ho in range(HO):
                    nc.sync.dma_start(w1t_s[:, ho], w1t_d[:, ho])
                    nc.sync.dma_start(featT_s[:, ho, 0:NCH], featT_d[:, ho, 0:NCH])
                nc.sync.dma_start(featT_s[:, :, NCH:FS], featT_d[:, :, NCH:FS])
                nc.sync.dma_start(w2t_s[:], w2t_d[:])

                for mo in range(HO):
                    kk_ps = psum.tile([P, KS], F32, tag="mm", bufs=6, name="kk_ps")
                    mm_accum(kk_ps[:], wkt_s, slice(mo * P, (mo + 1) * P),
                             kw_s, slice(0, KS))
                    nc.vector.tensor_scalar_add(
                        kbf_s[:, mo, :], kk_ps[:], bk_s[:, mo:mo + 1]
                    )
                    nc.scalar.activation(
                        sqk_s[:, mo, :], kk_ps[:], AF.Square,
                        bias=bk_s[:, mo:mo + 1], scale=1.0,
                    )

                # keyword norms (free-major) -> normalize -> fp8 -> AllGather
                ssum_k = emit_tree_sum(sqk_s, KS)
                nk_ps = psum.tile([1, KS], F32, tag="cn", bufs=1, name="nk_ps")
                nc.tensor.matmul(nk_ps[:], ones_s[:], ssum_k[:], start=True, stop=True)
                knr = work.tile([1, KS], F32, tag="knr", bufs=1, name="knr")
                # sqrt(nk / KSCALE^2) = ||k||/KSCALE; clamp; reciprocal
                nc.scalar.activation(
                    knr[:], nk_ps[:], AF.Sqrt, bias=0.0,
                    scale=1.0 / (KSCALE * KSCALE),
                )
                nc.vector.tensor_scalar_max(knr[:], knr[:], EPS / KSCALE)
                nc.vector.reciprocal(knr[:], knr[:])
                nc.gpsimd.partition_broadcast(rkn_b[:], knr[:])
                k8_stage = work.tile([P, HO, KS], FP8, tag="k8st", bufs=1, name="k8st")
                for ho in range(HO):
                    nc.vector.tensor_tensor(
                        k8_stage[:, ho, :], kbf_s[:, ho, :], rkn_b[:], ALU.mult
                    )
                nc.gpsimd.dma_start(cc_in[:], k8_stage[:])
                nc.gpsimd.collective_compute(
                    "AllGather",
                    mybir.AluOpType.bypass,
                    replica_groups=[list(range(NCORES))],
                    ins=[cc_in.opt()],
                    outs=[cc_out.opt()],
                )
                for r in range(NCORES):
                    nc.gpsimd.dma_start(
                        k8_all[:, :, r * KS:(r + 1) * KS], cc_out[r]
                    )

                # ---- MLP layer 1 ------------------------------------------
                for c in range(F_CHUNKS):
                    for mo in range(HO):
                        h1_ps = psum.tile([P, NCH], F32, tag="mm", bufs=6, name="h1_ps")
                        mm_accum(h1_ps[:], w1t_s, slice(mo * P, (mo + 1) * P),
                                 featT_s, slice(c * NCH, (c + 1) * NCH))
                        nc.scalar.activation(
                            hT_s[:, mo, c * NCH:(c + 1) * NCH],
                            h1_ps[:],
                            AF.Relu,
                            bias=b1_s[:, mo:mo + 1],
                            scale=1.0,
                        )

            # ---- MLP layer 2 + frame norms (partition-major) -------------
            for c in range(F_CHUNKS):
                sqf = work.tile([P, HO, NCH], BF16, tag="sqf", bufs=2, name="sqf")
                for mo in range(HO):
                    f2_ps = psum.tile([P, NCH], F32, tag="mm", bufs=6, name="f2_ps")
                    mm_accum(f2_ps[:], w2t_s, slice(mo * P, (mo + 1) * P),
                             hT_s, slice(c * NCH, (c + 1) * NCH))
                    nc.vector.tensor_scalar_add(
                        f8_s[:, mo, c * NCH:(c + 1) * NCH], f2_ps[:],
                        b2_s[:, mo:mo + 1],
                    )
                    nc.scalar.activation(
                        sqf[:, mo, :], f2_ps[:], AF.Square,
                        bias=b2_s[:, mo:mo + 1], scale=1.0,
                    )
                ssum_f = emit_tree_sum(sqf, NCH)
                for sub in range(NCH // P):
                    i = c * (NCH // P) + sub
                    nf_ps = psum.tile([P, 1], F32, tag="nrm", bufs=1, name="nf_ps")
                    nc.tensor.matmul(
                        nf_ps[:], ssum_f[:, sub * P:(sub + 1) * P], ones_s[:],
                        start=True, stop=True,
                    )
                    nc.scalar.copy(nf_raw[:, i:i + 1], nf_ps[:])

            # rfn = (0.49/KSCALE)/max(||f||, eps) = 1/max(||f||/C, eps/C)
            C = OUT_SCALE / KSCALE
            nc.scalar.activation(
                rfn_p[:], nf_raw[:], AF.Sqrt, bias=0.0, scale=1.0 / (C * C)
            )
            nc.vector.tensor_scalar_max(rfn_p[:], rfn_p[:], EPS / C)
            nc.vector.reciprocal(rfn_p[:], rfn_p[:])

            # ---- score GEMM (fp8 DoubleRow, frames stationary) -----------
            DR = mybir.MatmulPerfMode.DoubleRow
            NSTEP = HO // 2            # 4 contraction steps of 256
            KH = K_CHUNKS // 2         # 4 moving chunks per half
            for ft in range(F_TILES):
                fsl = slice(ft * P, (ft + 1) * P)
                for kh in range(2):
                    s_pss = [
                        psum.tile([P, NCH], F32, tag="mm", bufs=6, name="s_ps")
                        for _ in range(KH)
                    ]
                    for s in range(NSTEP):
                        lhs = f8_s[:, 2 * s:2 * s + 2, fsl]
                        for n4 in range(KH):
                            n = kh * KH + n4
                            rhs = k8_all[:, 2 * s:2 * s + 2, n * NCH:(n + 1) * NCH]
                            nc.tensor.matmul(
                                s_pss[n4][:], lhs, rhs,
                                start=(s == 0), stop=(s == NSTEP - 1),
                                perf_mode=DR,
                            )
                    for pair in range(KH // 2):
                        stage = work.tile([P, 2 * NCH], F32, tag="out_t", bufs=4,
                                          name="stage")
                        for half in range(2):
                            n4 = pair * 2 + half
                            nc.scalar.activation(
                                stage[:, half * NCH:(half + 1) * NCH],
                                s_pss[n4][:], AF.Identity,
                                bias=bias049_s[:, 0:1], scale=rfn_p[:, ft:ft + 1],
                            )
                        n0 = kh * KH + pair * 2
                        nc.sync.dma_start(
                            out_d[fsl, n0 * NCH:(n0 + 2) * NCH], stage[:]
                        )


def build():
    """Build + compile the (core-agnostic) Bass program once."""
    key = "nc_v5probe3"
    if key in _CACHE:
        return _CACHE[key]
    nc = bacc.Bacc(
        "TRN2",
        target_bir_lowering=False,
        debug=False,
        enable_asserts=False,
        num_devices=NCORES,
    )
    featT_d = nc.dram_tensor("featT", [P, HO, FS], BF16, kind="ExternalInput").ap()
    kwT_d = nc.dram_tensor("kwT", [P, HO, KS], BF16, kind="ExternalInput").ap()
    w1t_d = nc.dram_tensor("w1t", [P, HO, H], BF16, kind="ExternalInput").ap()
    w2t_d = nc.dram_tensor("w2t", [P, HO, H], BF16, kind="ExternalInput").ap()
    wkt_d = nc.dram_tensor("wkt", [P, HO, H], BF16, kind="ExternalInput").ap()
    b1_d = nc.dram_tensor("b1t", [P, HO], F32, kind="ExternalInput").ap()
    b2_d = nc.dram_tensor("b2t", [P, HO], F32, kind="ExternalInput").ap()
    bk_d = nc.dram_tensor("bkt", [P, HO], F32, kind="ExternalInput").ap()
    out_d = nc.dram_tensor("out", [FS, K], F32, kind="ExternalOutput").ap()

    io = (featT_d, kwT_d, w1t_d, w2t_d, wkt_d, b1_d, b2_d, bk_d, out_d)
    with tile.TileContext(nc) as tc:
        _emit(tc, io)
    nc.compile()
    _CACHE[key] = nc
    return nc


def _part_tile(a):
    """[D0, rest...] with D0 = o*P + p  ->  [P, D0//P, rest...]"""
    d0 = a.shape[0]
    return np.ascontiguousarray(
        a.reshape(d0 // P, P, *a.shape[1:]).swapaxes(0, 1)
    )


def make_in_maps(feat, keyword, W1, b1, W2, b2, Wk, bk):
    bf = ml_dtypes.bfloat16
    feat = np.asarray(feat, np.float32)
    keyword = np.asarray(keyword, np.float32)
    w1t = _part_tile(np.ascontiguousarray(np.asarray(W1, np.float32).T)).astype(bf)
    w2t = _part_tile(np.ascontiguousarray(np.asarray(W2, np.float32).T)).astype(bf)
    wkt = _part_tile(np.ascontiguousarray(np.asarray(Wk, np.float32).T)).astype(bf)
    b1t = _part_tile(np.asarray(b1, np.float32))                        # [P, HO]
    b2t = _part_tile(np.asarray(b2, np.float32))
    bkt = _part_tile(np.asarray(bk, np.float32))

    in_maps = []
    for c in range(NCORES):
        featT_c = _part_tile(
            np.ascontiguousarray(feat[c * FS:(c + 1) * FS, :].T)
        ).astype(bf)                                                    # [P, HO, FS]
        kwT_c = _part_tile(
            np.ascontiguousarray(keyword[c * KS:(c + 1) * KS, :].T)
        ).astype(bf)                                                    # [P, HO, KS]
        in_maps.append({
            "featT": featT_c,
            "kwT": kwT_c,
            "w1t": w1t,
            "w2t": w2t,
            "wkt": wkt,
            "b1t": b1t,
            "b2t": b2t,
            "bkt": bkt,
        })
    return in_maps


def assemble_out(shards):
    """shards[c] is the [FS, K] transposed score tile for frames of core c."""
    return np.ascontiguousarray(
        np.concatenate([np.asarray(s).T for s in shards], axis=1)
    ).astype(np.float32)


def kernel(feat, keyword, W1, b1, W2, b2, Wk, bk, _trace=False):
    global LAST_EXEC_NS, LAST_RESULTS
    nc = build()
    in_maps = make_in_maps(feat, keyword, W1, b1, W2, b2, Wk, bk)
    res = run_bass_kernel_spmd(
        nc,
        in_maps,
        core_ids=list(range(NCORES)),
        trace=_trace,
    )
    LAST_EXEC_NS = res.exec_time_ns
    LAST_RESULTS = res
    return assemble_out([res.results[c]["out"] for c in range(NCORES)])
